# revision 25
# baseline (speedup 1.0000x reference)
"""Trainium2 Bass kernel for per-batch adaptive 3D histogram binning + linear classifier.

reference semantics (per batch b):
    mins/maxs over N points per dim; scale = 8/rng
    idx = clip(floor((x-min)*scale), 0, 7) per dim
    flat = (idx0*8 + idx1)*8 + idx2  in [0, 512)
    counts = bincount(flat)/N ; logits = counts @ W.T + bias

Strategy (per core, data-parallel over batch across 8 cores):
  - 8 batches/core, points laid out [125 partitions x 800 cols]
  - phase 1: per-(batch,dim) min/max on DVE, then scale/bias table
    scb[ib*8+j]: j=0..3 scales (s0,s1,s2,s1/2), j=4..7 biases (+OFF)
  - binning via the fp16 round-to-nearest trick: ACT computes
    relu(s*x + t + 1088.501) -> fp16; fp16 RNE rounding on the integer-
    spaced grid [1024,2048) implements floor(u)+1089 directly (no int
    casts). hi = 4*i0 + floor(i1/2), lo = 8*(i1&1) + i2 derived with a
    few fp16 tensor_scalar/stt ops (fp32 internal arithmetic is exact).
  - one-hots via tensor_scalar is_equal at 4x DVE mode (16-bit dtypes,
    contiguous 8-wide inner runs); lo values 0..14 optionally offloaded
    to GPSIMD. Layout oh_lo[p,g,l,t], oh_hi[p,g,h,t] so matmul operands
    are single-stride contiguous (FWL stays on).
  - joint histogram: PSUM-accumulated matmuls over 8-column groups with
    the block-diagonal trick; psum row m = l*8+t, col f = h*8+t.
  - logits: counts (fp32 exact) @ (W/N) via a q=4-way folded matmul + b
"""

import os
import numpy as np
from contextlib import ExitStack

B_FULL = 64
N = 100000
CLASSES = 40
RES = 8
NBINS = RES**3  # 512
NCORES = 8
B_LOC = B_FULL // NCORES  # 8

P = 125            # SBUF partitions used for point data (125*800 = 100000)
COLS = N // P      # 800
GRP = 8            # columns per matmul group; psum partitions = GRP*LO = 128
NG = COLS // GRP   # 100 groups per batch
HI = 32
LO = 16

SCALE_EPS = 3e-4   # shrink scale so u(max) < 8 with margin for fp16 rounding
OFF = 1088.501     # fp16 round-floor offset: round(u+OFF) = 1089 + floor(u)
BASE = 1089.0

_CACHE = {}


def _iota_tables():
    import ml_dtypes
    il = np.repeat(np.arange(LO, dtype=np.float32), GRP).reshape(LO, GRP)
    ih = np.repeat(np.arange(HI, dtype=np.float32), GRP).reshape(HI, GRP)
    return (il.astype(ml_dtypes.bfloat16), ih.astype(ml_dtypes.bfloat16))


def _build_program():
    import concourse.bass as bass
    import concourse.bacc as bacc
    import concourse.tile as tile
    from concourse import mybir
    from concourse.masks import make_identity

    f32 = mybir.dt.float32
    f16 = mybir.dt.float16
    bf16 = mybir.dt.bfloat16
    Alu = mybir.AluOpType
    ActFn = mybir.ActivationFunctionType

    nc = bacc.Bacc(
        "TRN2",
        target_bir_lowering=False,
        debug=False,
        enable_asserts=False,
        num_devices=NCORES,
    )
    x_d = nc.dram_tensor("x", [B_LOC, N, 3], f32, kind="ExternalInput")
    w_d = nc.dram_tensor("W", [CLASSES, NBINS], f32, kind="ExternalInput")
    b_d = nc.dram_tensor("b", [CLASSES], f32, kind="ExternalInput")
    o_d = nc.dram_tensor("out", [B_LOC, CLASSES], f32, kind="ExternalOutput")
    s_d = nc.dram_tensor("scratch", [B_LOC, GRP * LO, GRP * HI], f32,
                         kind="Internal")
    il_d = nc.dram_tensor("iota_lo", [LO, GRP], bf16, kind="ExternalInput")
    ih_d = nc.dram_tensor("iota_hi", [HI, GRP], bf16, kind="ExternalInput")

    def reap(ap, dims, extra_offset=0):
        return bass.AP(tensor=ap.tensor, offset=ap.offset + extra_offset,
                       ap=dims)

    K3 = B_LOC * 3

    with tile.TileContext(nc) as tc, ExitStack() as ctx:
        consts = ctx.enter_context(tc.tile_pool(name="consts", bufs=1))
        xpool = ctx.enter_context(tc.tile_pool(name="xp", bufs=2))
        chain = ctx.enter_context(tc.tile_pool(name="ch", bufs=1))
        work = ctx.enter_context(tc.tile_pool(name="work", bufs=1))
        ohpool = ctx.enter_context(tc.tile_pool(name="oh", bufs=2))
        accum = ctx.enter_context(tc.tile_pool(name="acc", bufs=1))
        psum = ctx.enter_context(tc.tile_pool(name="ps", bufs=2, space="PSUM"))
        psum1 = ctx.enter_context(tc.tile_pool(name="ps1", bufs=2, space="PSUM"))

        x_ap = x_d.ap()
        w_ap = w_d.ap()
        b_ap = b_d.ap()
        o_ap = o_d.ap()

        ident = consts.tile([128, 128], f32)
        make_identity(nc, ident[:])
        ones_r = consts.tile([1, P], f32)
        nc.vector.memset(ones_r[:], 1.0)
        iota_lo = consts.tile([P, LO, GRP], bf16)   # value l at (l, t)
        nc.sync.dma_start(out=iota_lo[:],
                          in_=reap(il_d.ap(), [[0, P], [1, LO * GRP]]))
        iota_hi = consts.tile([P, HI, GRP], bf16)   # value h at (h, t)
        nc.sync.dma_start(out=iota_hi[:],
                          in_=reap(ih_d.ap(), [[0, P], [1, HI * GRP]]))

        # ------------- phase 1: per-(batch, dim) min / max ------------------
        # mmall cols [ib*3+d] = per-partition min, [32+ib*3+d] = max
        # x loads round-robin across engine DMA queues (single queue caps
        # at ~78 GB/s; the kernel moves 19.4 MB)
        dma_engs = [nc.sync, nc.gpsimd, nc.scalar]

        # split each load's partitions across the queues so the ~120ns/
        # descriptor fetch cost parallelizes (gpsimd's SWDGE path is the
        # slowest — give it the smallest slice)
        def load_x(ib, xt):
            xr = x_ap[ib].rearrange("(p c) d -> p c d", p=P)
            for eng, p0, p1 in ((nc.sync, 0, 63), (nc.scalar, 63, 125)):
                eng.dma_start(out=xt[p0:p1], in_=xr[p0:p1])

        mmall = accum.tile([P, 64], f32)
        nc.vector.memset(mmall[:], 0.0)
        for ib in range(B_LOC):
            xt = xpool.tile([P, COLS, 3], f32, tag="xt")
            load_x(ib, xt)
            xt_dc = xt[:].rearrange("p c d -> p d c")
            nc.vector.tensor_reduce(out=mmall[:, ib * 3:ib * 3 + 3], in_=xt_dc,
                                    axis=mybir.AxisListType.X, op=Alu.min)
            nc.vector.tensor_reduce(out=mmall[:, 32 + ib * 3:32 + ib * 3 + 3],
                                    in_=xt_dc,
                                    axis=mybir.AxisListType.X, op=Alu.max)

        # transpose [P, 64] -> [64, P], reduce across partitions -> [64, 1]
        tp1 = psum1.tile([64, 128], f32, tag="aux")
        nc.tensor.transpose(out=tp1[:, :P], in_=mmall[:],
                            identity=ident[:P, :P])
        mnmx = work.tile([64, 1], f32, tag="mnmx")
        nc.vector.memset(mnmx[:], 0.0)
        nc.vector.tensor_reduce(out=mnmx[:K3], in_=tp1[:K3, :P],
                                axis=mybir.AxisListType.X, op=Alu.min)
        nc.vector.tensor_reduce(out=mnmx[32:32 + K3], in_=tp1[32:32 + K3, :P],
                                axis=mybir.AxisListType.X, op=Alu.max)
        tp2 = psum1.tile([1, 64], f32, tag="aux")
        nc.tensor.transpose(out=tp2[:], in_=mnmx[:], identity=ident[:64, :64])
        tp2sb = work.tile([1, 64], f32, tag="tp2sb")
        nc.scalar.copy(out=tp2sb[:], in_=tp2[:])

        # scb row [1, 64]: col ib*8+j, j=0..2 s_d, 3 s1/2, 4..6 t_d, 7 t1h
        mn_v = reap(tp2sb[:], [tp2sb[:].ap[0], [3, B_LOC], [1, 3]])
        mx_v = reap(tp2sb[:], [tp2sb[:].ap[0], [3, B_LOC], [1, 3]],
                    extra_offset=32)
        rng_r = work.tile([1, K3], f32, tag="rng_r")
        rng_v = reap(rng_r[:], [rng_r[:].ap[0], [3, B_LOC], [1, 3]])
        nc.vector.tensor_tensor(out=rng_v, in0=mx_v, in1=mn_v,
                                op=Alu.subtract)
        rcp_r = work.tile([1, K3], f32, tag="rcp_r")
        nc.vector.reciprocal(out=rcp_r[:], in_=rng_r[:])

        scb = work.tile([1, 8 * B_LOC], f32, tag="scb")
        s_view = reap(scb[:], [scb[:].ap[0], [8, B_LOC], [1, 3]])
        nc.vector.tensor_scalar(
            out=s_view, in0=reap(rcp_r[:], [rcp_r[:].ap[0], [3, B_LOC], [1, 3]]),
            scalar1=float(RES) * (1.0 - SCALE_EPS), scalar2=None, op0=Alu.mult)
        # s1h = 0.5 * s1
        nc.vector.tensor_scalar(
            out=reap(scb[:], [scb[:].ap[0], [8, B_LOC]], extra_offset=3),
            in0=reap(scb[:], [scb[:].ap[0], [8, B_LOC]], extra_offset=1),
            scalar1=0.5, scalar2=None, op0=Alu.mult)
        # t_d = -mn_d * s_d + OFF
        tb = work.tile([1, K3], f32, tag="tb")
        tb_v = reap(tb[:], [tb[:].ap[0], [3, B_LOC], [1, 3]])
        nc.vector.scalar_tensor_tensor(out=tb_v, in0=mn_v, scalar=-1.0,
                                       in1=s_view, op0=Alu.mult, op1=Alu.mult)
        nc.vector.tensor_scalar(
            out=reap(scb[:], [scb[:].ap[0], [8, B_LOC], [1, 3]],
                     extra_offset=4),
            in0=tb_v, scalar1=OFF, scalar2=None, op0=Alu.add)
        # t1h = -mn1 * s1h + OFF
        tb2 = work.tile([1, B_LOC], f32, tag="tb2")
        nc.vector.scalar_tensor_tensor(
            out=tb2[:],
            in0=reap(tp2sb[:], [tp2sb[:].ap[0], [3, B_LOC]], extra_offset=1),
            scalar=-1.0,
            in1=reap(scb[:], [scb[:].ap[0], [8, B_LOC]], extra_offset=3),
            op0=Alu.mult, op1=Alu.mult)
        nc.vector.tensor_scalar(
            out=reap(scb[:], [scb[:].ap[0], [8, B_LOC]], extra_offset=7),
            in0=tb2[:], scalar1=OFF, scalar2=None, op0=Alu.add)

        # broadcast to all P partitions
        tp3 = psum1.tile([P, 8 * B_LOC], f32, tag="aux")
        nc.tensor.matmul(out=tp3[:], lhsT=ones_r[:], rhs=scb[:],
                         start=True, stop=True)
        scbb = accum.tile([P, 8 * B_LOC], f32)
        nc.scalar.copy(out=scbb[:], in_=tp3[:])

        # ------------- phase 2: binning + one-hots + histogram --------------
        counts_all = accum.tile([LO, B_LOC, HI], f32)
        for ib in range(B_LOC):
            xt = xpool.tile([P, COLS, 3], f32, tag="xt")
            load_x(ib, xt)

            # w planes: j=0: i0+B, j=1: i1+B, j=2: i2+B, j=3: floor(u1/2)+B
            w = chain.tile([P, 4, COLS], f16, tag="w")
            for j, d in ((0, 0), (1, 1), (2, 2), (3, 1)):
                nc.scalar.activation(
                    out=w[:, j, :], in_=xt[:, :, d], func=ActFn.Relu,
                    bias=scbb[:, ib * 8 + 4 + j:ib * 8 + 5 + j],
                    scale=scbb[:, ib * 8 + j:ib * 8 + 1 + j])

            # hi = 4*i0 + b1 ; lo = 8*i1 + i2 - 16*b1   (fp32 internal, exact;
            # values are small ints so bf16 outputs are exact)
            a0 = chain.tile([P, COLS], bf16, tag="a0")
            nc.vector.tensor_scalar(out=a0[:], in0=w[:, 0, :], scalar1=4.0,
                                    scalar2=-4.0 * BASE, op0=Alu.mult,
                                    op1=Alu.add)
            b1v = chain.tile([P, COLS], bf16, tag="b1v")
            nc.vector.tensor_scalar(out=b1v[:], in0=w[:, 3, :], scalar1=-BASE,
                                    scalar2=None, op0=Alu.add)
            hi_v = chain.tile([P, COLS], bf16, tag="hi_v")
            nc.vector.tensor_tensor(out=hi_v[:], in0=a0[:], in1=b1v[:],
                                    op=Alu.add)
            a1 = chain.tile([P, COLS], bf16, tag="a1")
            nc.vector.tensor_scalar(out=a1[:], in0=w[:, 1, :], scalar1=8.0,
                                    scalar2=-8.0 * BASE, op0=Alu.mult,
                                    op1=Alu.add)
            z = chain.tile([P, COLS], bf16, tag="z")
            nc.vector.scalar_tensor_tensor(out=z[:], in0=w[:, 2, :],
                                           scalar=-BASE, in1=a1[:],
                                           op0=Alu.add, op1=Alu.add)
            lo_v = chain.tile([P, COLS], bf16, tag="lo_v")
            nc.vector.scalar_tensor_tensor(out=lo_v[:], in0=b1v[:],
                                           scalar=-16.0, in1=z[:],
                                           op0=Alu.mult, op1=Alu.add)

            # one-hots in grouped layout, TT is_equal vs materialized iota
            # tables: both inputs step-1 innermost -> 2x DVE mode.
            # oh_lo[p, g, l, t] = (lo_v[p, g*8+t] == l), weight col m=l*8+t
            oh_lo = ohpool.tile([P, NG, LO, GRP], bf16, tag="oh_lo")
            oh_hi = ohpool.tile([P, NG, HI, GRP], bf16, tag="oh_hi")
            nc.vector.tensor_tensor(
                out=reap(oh_lo[:], [oh_lo[:].ap[0], [LO * GRP, NG],
                                    [1, LO * GRP]]),
                in0=reap(lo_v[:], [lo_v[:].ap[0], [GRP, NG], [0, LO],
                                   [1, GRP]]),
                in1=reap(iota_lo[:], [iota_lo[:].ap[0], [0, NG],
                                      [1, LO * GRP]]),
                op=Alu.is_equal)
            nc.vector.tensor_tensor(
                out=reap(oh_hi[:], [oh_hi[:].ap[0], [HI * GRP, NG],
                                    [1, HI * GRP]]),
                in0=reap(hi_v[:], [hi_v[:].ap[0], [GRP, NG], [0, HI],
                                   [1, GRP]]),
                in1=reap(iota_hi[:], [iota_hi[:].ap[0], [0, NG],
                                      [1, HI * GRP]]),
                op=Alu.is_equal)

            # histogram matmuls: psum[m=(l,t), f=(h,t')], contiguous operands
            pt = psum.tile([GRP * LO, GRP * HI], f32, tag="pt")
            for g in range(NG):
                lhsT = reap(oh_lo[:], [oh_lo[:].ap[0], [1, LO * GRP]],
                            extra_offset=g * LO * GRP)
                rhs = reap(oh_hi[:], [oh_hi[:].ap[0], [1, HI * GRP]],
                           extra_offset=g * HI * GRP)
                nc.tensor.matmul(out=pt[:], lhsT=lhsT, rhs=rhs,
                                 start=(g == 0), stop=(g == NG - 1))

            # gather diagonal blocks: (l, h) at pt[l*8+t, h*8+t].
            # partition-strided SBUF reads trip the sim's conflict checker,
            # so bounce through DRAM where the AP is flat:
            # flat idx = (l*8+t)*256 + h*8+t = l*2048 + t*257 + h*8
            ptsb = work.tile([GRP * LO, GRP * HI], f32, tag="ptsb")
            nc.scalar.copy(out=ptsb[:], in_=pt[:])
            nc.sync.dma_start(out=s_d.ap()[ib], in_=ptsb[:])
            diag = work.tile([LO, GRP, HI], f32, tag="diag")
            for t in range(GRP):
                nc.sync.dma_start(
                    out=diag[:, t, :],
                    in_=reap(s_d.ap()[ib], [[GRP * GRP * HI, LO], [GRP, HI]],
                             extra_offset=t * (GRP * HI + 1)))
            nc.vector.tensor_reduce(out=counts_all[:, ib, :],
                                    in_=diag[:].rearrange("l t h -> l h t"),
                                    axis=mybir.AxisListType.X, op=Alu.add)

        # ------------- final: logits = counts/N @ W.T + b -------------------
        # cnt128[pp, q, ib] = counts(lo=pp%16, hi=q*8+pp//16) of batch ib
        # (flat = q*128 + pp), Wr[pp, q, c] = W[c, q*128+pp] / N
        cnt128 = accum.tile([GRP * LO, 4, B_LOC], f32)
        for j in range(8):
            for q in range(4):
                src = reap(counts_all[:],
                           [counts_all[:].ap[0], [HI, B_LOC]],
                           extra_offset=j + q * 8)
                nc.sync.dma_start(out=cnt128[j * 16:(j + 1) * 16, q, :],
                                  in_=src)

        wr = accum.tile([GRP * LO, 4, CLASSES], f32)
        for q in range(4):
            w_src = reap(w_ap, [[1, 128], [NBINS, CLASSES]],
                         extra_offset=q * 128)
            nc.sync.dma_start(out=wr[:, q, :], in_=w_src)
        wrs = accum.tile([GRP * LO, 4, CLASSES], f32)
        nc.vector.tensor_scalar(out=wrs[:], in0=wr[:], scalar1=1.0 / N,
                                scalar2=None, op0=Alu.mult)

        pf = psum1.tile([4 * B_LOC, 4 * CLASSES], f32, tag="aux")
        nc.tensor.matmul(out=pf[:], lhsT=cnt128[:].rearrange("p q b -> p (q b)"),
                         rhs=wrs[:].rearrange("p q c -> p (q c)"),
                         start=True, stop=True)

        pfsb = work.tile([4 * B_LOC, 4 * CLASSES], f32, tag="pfsb")
        nc.scalar.copy(out=pfsb[:], in_=pf[:])
        fin = work.tile([B_LOC, 4, CLASSES], f32, tag="fin")
        for q in range(4):
            nc.sync.dma_start(out=fin[:, q, :],
                              in_=pfsb[q * B_LOC:(q + 1) * B_LOC,
                                       q * CLASSES:(q + 1) * CLASSES])
        biast = work.tile([B_LOC, CLASSES], f32, tag="biast")
        nc.sync.dma_start(out=biast[:],
                          in_=reap(b_ap, [[0, B_LOC], [1, CLASSES]]))
        red = work.tile([B_LOC, CLASSES], f32, tag="red")
        nc.vector.tensor_reduce(out=red[:],
                                in_=fin[:].rearrange("b q c -> b c q"),
                                axis=mybir.AxisListType.X, op=Alu.add)
        logits = work.tile([B_LOC, CLASSES], f32, tag="logits")
        nc.vector.tensor_tensor(out=logits[:], in0=red[:], in1=biast[:],
                                op=Alu.add)
        nc.sync.dma_start(out=o_ap, in_=logits[:])

    nc.compile()
    return nc


def _get_program():
    if "nc" not in _CACHE:
        _CACHE["nc"] = _build_program()
    return _CACHE["nc"]


class _Runner:
    """Cached jit(shard_map(bass_exec)) fast path (mirrors
    concourse.bass2jax.run_bass_via_pjrt, but built once and reused)."""

    def __init__(self, nc):
        import jax
        from jax.sharding import Mesh, PartitionSpec, NamedSharding
        from jax.experimental.shard_map import shard_map
        from concourse import mybir
        from concourse.bass2jax import (
            _bass_exec_p,
            partition_id_tensor,
            install_neuronx_cc_hook,
        )

        install_neuronx_cc_hook()
        self.jax = jax
        self.nc = nc
        pname = nc.partition_id_tensor.name if nc.partition_id_tensor else None
        in_names, out_names, out_avals, zero_outs = [], [], [], []
        for alloc in nc.m.functions[0].allocations:
            if not isinstance(alloc, mybir.MemoryLocationSet):
                continue
            name = alloc.memorylocations[0].name
            if alloc.kind == "ExternalInput":
                if name != pname:
                    in_names.append(name)
            elif alloc.kind == "ExternalOutput":
                shape = tuple(alloc.tensor_shape)
                dtype = mybir.dt.np(alloc.dtype)
                out_avals.append(jax.core.ShapedArray(shape, dtype))
                out_names.append(name)
                zero_outs.append(np.zeros(shape, dtype))
        n_params = len(in_names)
        n_outs = len(out_names)
        all_in_names = in_names + out_names
        if pname is not None:
            all_in_names.append(pname)
        self.in_names = in_names
        self.out_names = out_names
        self.out_avals = out_avals
        self.zero_outs = zero_outs
        self.n_params = n_params

        def _body(*args):
            operands = list(args)
            if pname is not None:
                operands.append(partition_id_tensor())
            outs = _bass_exec_p.bind(
                *operands,
                out_avals=tuple(out_avals),
                in_names=tuple(all_in_names),
                out_names=tuple(out_names),
                lowering_input_output_aliases=(),
                sim_require_finite=True,
                sim_require_nnan=True,
                nc=nc,
            )
            return tuple(outs)

        devices = jax.devices()[:NCORES]
        self.mesh = Mesh(np.asarray(devices), ("core",))
        in_specs = (PartitionSpec("core"),) * (n_params + n_outs)
        out_specs = (PartitionSpec("core"),) * n_outs
        self.sharding = NamedSharding(self.mesh, PartitionSpec("core"))
        donate = tuple(range(n_params, n_params + n_outs))
        self.fn = jax.jit(
            shard_map(_body, mesh=self.mesh, in_specs=in_specs,
                      out_specs=out_specs, check_rep=False),
            donate_argnums=donate, keep_unused=True,
        )

    def concat_inputs(self, in_maps):
        return [
            np.concatenate([np.asarray(m[name]) for m in in_maps], axis=0)
            for name in self.in_names
        ]

    def device_put_inputs(self, in_maps):
        return [
            self.jax.device_put(a, self.sharding)
            for a in self.concat_inputs(in_maps)
        ]

    def call(self, concat_in):
        zeros = [
            np.zeros((NCORES * z.shape[0], *z.shape[1:]), z.dtype)
            for z in self.zero_outs
        ]
        return self.fn(*concat_in, *zeros)

    def run(self, in_maps):
        out_arrs = self.call(self.concat_inputs(in_maps))
        return [
            {
                name: np.asarray(out_arrs[i]).reshape(
                    NCORES, *self.out_avals[i].shape)[c]
                for i, name in enumerate(self.out_names)
            }
            for c in range(NCORES)
        ]


def _make_runner():
    if "runner" not in _CACHE:
        _CACHE["runner"] = _Runner(_get_program())
    return _CACHE["runner"]


def _in_maps(x, W, b):
    il, ih = _iota_tables()
    return [
        {
            "x": np.ascontiguousarray(x[i * B_LOC:(i + 1) * B_LOC]),
            "W": np.ascontiguousarray(W),
            "b": np.ascontiguousarray(b),
            "iota_lo": il,
            "iota_hi": ih,
        }
        for i in range(NCORES)
    ]


def _run(x, W, b, trace=False, trace_cores=None):
    from concourse.bass_utils import run_bass_kernel_spmd

    nc = _get_program()
    return run_bass_kernel_spmd(nc, _in_maps(x, W, b),
                                core_ids=list(range(NCORES)), trace=trace,
                                trace_cores=trace_cores)


def kernel(**inputs):
    x = inputs["x"]
    W = inputs["W"]
    b = inputs["b"]
    assert x.shape == (B_FULL, N, 3) and x.dtype == np.float32
    res = _make_runner().run(_in_maps(x, W, b))
    return np.concatenate([r["out"] for r in res], axis=0)


# revision 28
# speedup vs baseline: 1.0806x; 1.0806x over previous
"""Trainium2 Bass kernel for per-batch adaptive 3D histogram binning + linear classifier.

reference semantics (per batch b):
    mins/maxs over N points per dim; scale = 8/rng
    idx = clip(floor((x-min)*scale), 0, 7) per dim
    flat = (idx0*8 + idx1)*8 + idx2  in [0, 512)
    counts = bincount(flat)/N ; logits = counts @ W.T + bias

Strategy (per core, data-parallel over batch across 8 cores):
  - 8 batches/core, points laid out [125 partitions x 800 cols]
  - phase 1: per-(batch,dim) min/max on DVE, then scale/bias table
    scb[ib*8+j]: j=0..3 scales (s0,s1,s2,s1/2), j=4..7 biases (+OFF)
  - binning via the fp16 round-to-nearest trick: ACT computes
    relu(s*x + t + 1088.501) -> fp16; fp16 RNE rounding on the integer-
    spaced grid [1024,2048) implements floor(u)+1089 directly (no int
    casts). hi = 4*i0 + floor(i1/2), lo = 8*(i1&1) + i2 derived with a
    few fp16 tensor_scalar/stt ops (fp32 internal arithmetic is exact).
  - one-hots via tensor_scalar is_equal at 4x DVE mode (16-bit dtypes,
    contiguous 8-wide inner runs); lo values 0..14 optionally offloaded
    to GPSIMD. Layout oh_lo[p,g,l,t], oh_hi[p,g,h,t] so matmul operands
    are single-stride contiguous (FWL stays on).
  - joint histogram: PSUM-accumulated matmuls over 8-column groups with
    the block-diagonal trick; psum row m = l*8+t, col f = h*8+t.
  - logits: counts (fp32 exact) @ (W/N) via a q=4-way folded matmul + b
"""

import os
import numpy as np
from contextlib import ExitStack

B_FULL = 64
N = 100000
CLASSES = 40
RES = 8
NBINS = RES**3  # 512
NCORES = 8
B_LOC = B_FULL // NCORES  # 8

P = 125            # SBUF partitions used for point data (125*800 = 100000)
COLS = N // P      # 800
GRP = 8            # columns per matmul group; psum partitions = GRP*LO = 128
NG = COLS // GRP   # 100 groups per batch
HI = 32
LO = 16

SCALE_EPS = 3e-4   # shrink scale so u(max) < 8 with margin for fp16 rounding
OFF = 1088.501     # fp16 round-floor offset: round(u+OFF) = 1089 + floor(u)
BASE = 1089.0

_CACHE = {}


def _iota_tables():
    import ml_dtypes
    il = np.repeat(np.arange(LO, dtype=np.float32), GRP).reshape(LO, GRP)
    ih = np.repeat(np.arange(HI, dtype=np.float32), GRP).reshape(HI, GRP)
    return (il.astype(ml_dtypes.bfloat16), ih.astype(ml_dtypes.bfloat16))


def _build_program():
    import concourse.bass as bass
    import concourse.bacc as bacc
    import concourse.tile as tile
    from concourse import mybir
    from concourse.masks import make_identity

    f32 = mybir.dt.float32
    f16 = mybir.dt.float16
    bf16 = mybir.dt.bfloat16
    Alu = mybir.AluOpType
    ActFn = mybir.ActivationFunctionType

    nc = bacc.Bacc(
        "TRN2",
        target_bir_lowering=False,
        debug=False,
        enable_asserts=False,
        num_devices=NCORES,
    )
    x_d = nc.dram_tensor("x", [B_LOC, N, 3], f32, kind="ExternalInput")
    w_d = nc.dram_tensor("W", [CLASSES, NBINS], f32, kind="ExternalInput")
    b_d = nc.dram_tensor("b", [CLASSES], f32, kind="ExternalInput")
    o_d = nc.dram_tensor("out", [B_LOC, CLASSES], f32, kind="ExternalOutput")
    s_d = nc.dram_tensor("scratch", [B_LOC, GRP * LO, GRP * HI], f32,
                         kind="Internal")
    il_d = nc.dram_tensor("iota_lo", [LO, GRP], bf16, kind="ExternalInput")
    ih_d = nc.dram_tensor("iota_hi", [HI, GRP], bf16, kind="ExternalInput")

    def reap(ap, dims, extra_offset=0):
        return bass.AP(tensor=ap.tensor, offset=ap.offset + extra_offset,
                       ap=dims)

    K3 = B_LOC * 3

    with tile.TileContext(nc) as tc, ExitStack() as ctx:
        consts = ctx.enter_context(tc.tile_pool(name="consts", bufs=1))
        xpool = ctx.enter_context(tc.tile_pool(name="xp", bufs=2))
        chain = ctx.enter_context(tc.tile_pool(name="ch", bufs=1))
        work = ctx.enter_context(tc.tile_pool(name="work", bufs=1))
        ohpool = ctx.enter_context(tc.tile_pool(name="oh", bufs=2))
        accum = ctx.enter_context(tc.tile_pool(name="acc", bufs=1))
        psum = ctx.enter_context(tc.tile_pool(name="ps", bufs=2, space="PSUM"))
        psum1 = ctx.enter_context(tc.tile_pool(name="ps1", bufs=2, space="PSUM"))

        x_ap = x_d.ap()
        w_ap = w_d.ap()
        b_ap = b_d.ap()
        o_ap = o_d.ap()

        ident = consts.tile([128, 128], f32)
        make_identity(nc, ident[:])
        ones_r = consts.tile([1, P], f32)
        nc.vector.memset(ones_r[:], 1.0)
        iota_lo = consts.tile([P, LO, GRP], bf16)   # value l at (l, t)
        nc.sync.dma_start(out=iota_lo[:],
                          in_=reap(il_d.ap(), [[0, P], [1, LO * GRP]]))
        iota_hi = consts.tile([P, HI, GRP], bf16)   # value h at (h, t)
        nc.sync.dma_start(out=iota_hi[:],
                          in_=reap(ih_d.ap(), [[0, P], [1, HI * GRP]]))

        # ------------- phase 1: per-(batch, dim) min / max ------------------
        # mmall cols [ib*3+d] = per-partition min, [32+ib*3+d] = max
        # x loads round-robin across engine DMA queues (single queue caps
        # at ~78 GB/s; the kernel moves 19.4 MB)
        dma_engs = [nc.sync, nc.gpsimd, nc.scalar]

        # split each load's partitions across the queues so the ~120ns/
        # descriptor fetch cost parallelizes (gpsimd's SWDGE path is the
        # slowest — give it the smallest slice)
        def load_x(ib, xt):
            xr = x_ap[ib].rearrange("(p c) d -> p c d", p=P)
            for eng, p0, p1 in ((nc.sync, 0, 50), (nc.scalar, 50, 100),
                                (nc.gpsimd, 100, 125)):
                eng.dma_start(out=xt[p0:p1], in_=xr[p0:p1])

        scbb = accum.tile([P, 8 * B_LOC], f32)

        # ------------- fused loop: minmax + scb + binning per batch ---------
        # (x loaded once per batch; batch ib's reductions overlap batch
        # ib-1's one-hots/matmuls)
        counts_all = accum.tile([LO, B_LOC, HI], f32)
        for ib in range(B_LOC):
            xt = xpool.tile([P, COLS, 3], f32, tag="xt")
            load_x(ib, xt)

            # per-(partition, dim) min/max -> [1, 8] row: cols 0..2 mn,
            # 4..6 mx (3, 7 padding)
            mm8 = work.tile([P, 36], f32, tag="mm8")
            nc.vector.memset(mm8[:], 0.0)
            xt_dc = xt[:].rearrange("p c d -> p d c")
            nc.vector.tensor_reduce(out=mm8[:, 0:3], in_=xt_dc,
                                    axis=mybir.AxisListType.X, op=Alu.min)
            nc.vector.tensor_reduce(out=mm8[:, 32:35], in_=xt_dc,
                                    axis=mybir.AxisListType.X, op=Alu.max)
            tp1 = psum1.tile([36, 128], f32, tag="aux")
            nc.tensor.transpose(out=tp1[:, :P], in_=mm8[:],
                                identity=ident[:P, :P])
            mnb = work.tile([36, 1], f32, tag="mnb")
            nc.vector.memset(mnb[:], 0.0)
            nc.vector.tensor_reduce(out=mnb[0:3], in_=tp1[0:3, :P],
                                    axis=mybir.AxisListType.X, op=Alu.min)
            nc.vector.tensor_reduce(out=mnb[32:35], in_=tp1[32:35, :P],
                                    axis=mybir.AxisListType.X, op=Alu.max)
            tp2 = psum1.tile([1, 36], f32, tag="aux")
            nc.tensor.transpose(out=tp2[:], in_=mnb[:], identity=ident[:36, :36])
            row = work.tile([1, 36], f32, tag="rowb")
            nc.scalar.copy(out=row[:], in_=tp2[:])

            # scb slice [1, 8]: j=0..2 s_d, 3 s1/2, 4..6 t_d, 7 t1h
            rngb = work.tile([1, 3], f32, tag="rngb")
            nc.vector.tensor_tensor(out=rngb[:], in0=row[:, 32:35],
                                    in1=row[:, 0:3], op=Alu.subtract)
            rcpb = work.tile([1, 3], f32, tag="rcpb")
            nc.vector.reciprocal(out=rcpb[:], in_=rngb[:])
            scr = work.tile([1, 8], f32, tag="scr")
            nc.vector.tensor_scalar(
                out=scr[:, 0:3], in0=rcpb[:],
                scalar1=float(RES) * (1.0 - SCALE_EPS), scalar2=None,
                op0=Alu.mult)
            nc.vector.tensor_scalar(out=scr[:, 3:4], in0=scr[:, 1:2],
                                    scalar1=0.5, scalar2=None, op0=Alu.mult)
            tbb = work.tile([1, 3], f32, tag="tbb")
            nc.vector.scalar_tensor_tensor(out=tbb[:], in0=row[:, 0:3],
                                           scalar=-1.0, in1=scr[:, 0:3],
                                           op0=Alu.mult, op1=Alu.mult)
            nc.vector.tensor_scalar(out=scr[:, 4:7], in0=tbb[:], scalar1=OFF,
                                    scalar2=None, op0=Alu.add)
            tb2b = work.tile([1, 1], f32, tag="tb2b")
            nc.vector.scalar_tensor_tensor(out=tb2b[:], in0=row[:, 1:2],
                                           scalar=-1.0, in1=scr[:, 3:4],
                                           op0=Alu.mult, op1=Alu.mult)
            nc.vector.tensor_scalar(out=scr[:, 7:8], in0=tb2b[:], scalar1=OFF,
                                    scalar2=None, op0=Alu.add)
            tp3 = psum1.tile([P, 8], f32, tag="aux")
            nc.tensor.matmul(out=tp3[:], lhsT=ones_r[:], rhs=scr[:],
                             start=True, stop=True)
            nc.scalar.copy(out=scbb[:, ib * 8:(ib + 1) * 8], in_=tp3[:])

            # w planes: j=0: i0+B, j=1: i1+B, j=2: i2+B, j=3: floor(u1/2)+B
            w = chain.tile([P, 4, COLS], f16, tag="w")
            for j, d in ((0, 0), (1, 1), (2, 2), (3, 1)):
                nc.scalar.activation(
                    out=w[:, j, :], in_=xt[:, :, d], func=ActFn.Relu,
                    bias=scbb[:, ib * 8 + 4 + j:ib * 8 + 5 + j],
                    scale=scbb[:, ib * 8 + j:ib * 8 + 1 + j])

            # hi = 4*i0 + b1 ; lo = 8*i1 + i2 - 16*b1   (fp32 internal, exact;
            # values are small ints so bf16 outputs are exact)
            a0 = chain.tile([P, COLS], bf16, tag="a0")
            nc.vector.tensor_scalar(out=a0[:], in0=w[:, 0, :], scalar1=4.0,
                                    scalar2=-4.0 * BASE, op0=Alu.mult,
                                    op1=Alu.add)
            b1v = chain.tile([P, COLS], bf16, tag="b1v")
            nc.vector.tensor_scalar(out=b1v[:], in0=w[:, 3, :], scalar1=-BASE,
                                    scalar2=None, op0=Alu.add)
            hi_v = chain.tile([P, COLS], bf16, tag="hi_v")
            nc.vector.tensor_tensor(out=hi_v[:], in0=a0[:], in1=b1v[:],
                                    op=Alu.add)
            a1 = chain.tile([P, COLS], bf16, tag="a1")
            nc.vector.tensor_scalar(out=a1[:], in0=w[:, 1, :], scalar1=8.0,
                                    scalar2=-8.0 * BASE, op0=Alu.mult,
                                    op1=Alu.add)
            z = chain.tile([P, COLS], bf16, tag="z")
            nc.vector.scalar_tensor_tensor(out=z[:], in0=w[:, 2, :],
                                           scalar=-BASE, in1=a1[:],
                                           op0=Alu.add, op1=Alu.add)
            lo_v = chain.tile([P, COLS], bf16, tag="lo_v")
            nc.vector.scalar_tensor_tensor(out=lo_v[:], in0=b1v[:],
                                           scalar=-16.0, in1=z[:],
                                           op0=Alu.mult, op1=Alu.add)

            # one-hots in grouped layout, TT is_equal vs materialized iota
            # tables: both inputs step-1 innermost -> 2x DVE mode.
            # oh_lo[p, g, l, t] = (lo_v[p, g*8+t] == l), weight col m=l*8+t
            oh_lo = ohpool.tile([P, NG, LO, GRP], bf16, tag="oh_lo")
            oh_hi = ohpool.tile([P, NG, HI, GRP], bf16, tag="oh_hi")
            nc.vector.tensor_tensor(
                out=reap(oh_lo[:], [oh_lo[:].ap[0], [LO * GRP, NG],
                                    [1, LO * GRP]]),
                in0=reap(lo_v[:], [lo_v[:].ap[0], [GRP, NG], [0, LO],
                                   [1, GRP]]),
                in1=reap(iota_lo[:], [iota_lo[:].ap[0], [0, NG],
                                      [1, LO * GRP]]),
                op=Alu.is_equal)
            nc.vector.tensor_tensor(
                out=reap(oh_hi[:], [oh_hi[:].ap[0], [HI * GRP, NG],
                                    [1, HI * GRP]]),
                in0=reap(hi_v[:], [hi_v[:].ap[0], [GRP, NG], [0, HI],
                                   [1, GRP]]),
                in1=reap(iota_hi[:], [iota_hi[:].ap[0], [0, NG],
                                      [1, HI * GRP]]),
                op=Alu.is_equal)

            # histogram matmuls: psum[m=(l,t), f=(h,t')], contiguous operands
            pt = psum.tile([GRP * LO, GRP * HI], f32, tag="pt")
            for g in range(NG):
                lhsT = reap(oh_lo[:], [oh_lo[:].ap[0], [1, LO * GRP]],
                            extra_offset=g * LO * GRP)
                rhs = reap(oh_hi[:], [oh_hi[:].ap[0], [1, HI * GRP]],
                           extra_offset=g * HI * GRP)
                nc.tensor.matmul(out=pt[:], lhsT=lhsT, rhs=rhs,
                                 start=(g == 0), stop=(g == NG - 1))

            # gather diagonal blocks: (l, h) at pt[l*8+t, h*8+t].
            # partition-strided SBUF reads trip the sim's conflict checker,
            # so bounce through DRAM where the AP is flat:
            # flat idx = (l*8+t)*256 + h*8+t = l*2048 + t*257 + h*8
            ptsb = work.tile([GRP * LO, GRP * HI], f32, tag="ptsb")
            nc.scalar.copy(out=ptsb[:], in_=pt[:])
            nc.sync.dma_start(out=s_d.ap()[ib], in_=ptsb[:])
            diag = work.tile([LO, GRP, HI], f32, tag="diag")
            for t in range(GRP):
                nc.sync.dma_start(
                    out=diag[:, t, :],
                    in_=reap(s_d.ap()[ib], [[GRP * GRP * HI, LO], [GRP, HI]],
                             extra_offset=t * (GRP * HI + 1)))
            nc.vector.tensor_reduce(out=counts_all[:, ib, :],
                                    in_=diag[:].rearrange("l t h -> l h t"),
                                    axis=mybir.AxisListType.X, op=Alu.add)

        # ------------- final: logits = counts/N @ W.T + b -------------------
        # cnt128[pp, q, ib] = counts(lo=pp%16, hi=q*8+pp//16) of batch ib
        # (flat = q*128 + pp), Wr[pp, q, c] = W[c, q*128+pp] / N
        cnt128 = accum.tile([GRP * LO, 4, B_LOC], f32)
        for j in range(8):
            for q in range(4):
                src = reap(counts_all[:],
                           [counts_all[:].ap[0], [HI, B_LOC]],
                           extra_offset=j + q * 8)
                nc.sync.dma_start(out=cnt128[j * 16:(j + 1) * 16, q, :],
                                  in_=src)

        wr = accum.tile([GRP * LO, 4, CLASSES], f32)
        for q in range(4):
            w_src = reap(w_ap, [[1, 128], [NBINS, CLASSES]],
                         extra_offset=q * 128)
            nc.sync.dma_start(out=wr[:, q, :], in_=w_src)
        wrs = accum.tile([GRP * LO, 4, CLASSES], f32)
        nc.vector.tensor_scalar(out=wrs[:], in0=wr[:], scalar1=1.0 / N,
                                scalar2=None, op0=Alu.mult)

        pf = psum1.tile([4 * B_LOC, 4 * CLASSES], f32, tag="aux")
        nc.tensor.matmul(out=pf[:], lhsT=cnt128[:].rearrange("p q b -> p (q b)"),
                         rhs=wrs[:].rearrange("p q c -> p (q c)"),
                         start=True, stop=True)

        pfsb = work.tile([4 * B_LOC, 4 * CLASSES], f32, tag="pfsb")
        nc.scalar.copy(out=pfsb[:], in_=pf[:])
        fin = work.tile([B_LOC, 4, CLASSES], f32, tag="fin")
        for q in range(4):
            nc.sync.dma_start(out=fin[:, q, :],
                              in_=pfsb[q * B_LOC:(q + 1) * B_LOC,
                                       q * CLASSES:(q + 1) * CLASSES])
        biast = work.tile([B_LOC, CLASSES], f32, tag="biast")
        nc.sync.dma_start(out=biast[:],
                          in_=reap(b_ap, [[0, B_LOC], [1, CLASSES]]))
        red = work.tile([B_LOC, CLASSES], f32, tag="red")
        nc.vector.tensor_reduce(out=red[:],
                                in_=fin[:].rearrange("b q c -> b c q"),
                                axis=mybir.AxisListType.X, op=Alu.add)
        logits = work.tile([B_LOC, CLASSES], f32, tag="logits")
        nc.vector.tensor_tensor(out=logits[:], in0=red[:], in1=biast[:],
                                op=Alu.add)
        nc.sync.dma_start(out=o_ap, in_=logits[:])

    nc.compile()
    return nc


def _get_program():
    if "nc" not in _CACHE:
        _CACHE["nc"] = _build_program()
    return _CACHE["nc"]


class _Runner:
    """Cached jit(shard_map(bass_exec)) fast path (mirrors
    concourse.bass2jax.run_bass_via_pjrt, but built once and reused)."""

    def __init__(self, nc):
        import jax
        from jax.sharding import Mesh, PartitionSpec, NamedSharding
        from jax.experimental.shard_map import shard_map
        from concourse import mybir
        from concourse.bass2jax import (
            _bass_exec_p,
            partition_id_tensor,
            install_neuronx_cc_hook,
        )

        install_neuronx_cc_hook()
        self.jax = jax
        self.nc = nc
        pname = nc.partition_id_tensor.name if nc.partition_id_tensor else None
        in_names, out_names, out_avals, zero_outs = [], [], [], []
        for alloc in nc.m.functions[0].allocations:
            if not isinstance(alloc, mybir.MemoryLocationSet):
                continue
            name = alloc.memorylocations[0].name
            if alloc.kind == "ExternalInput":
                if name != pname:
                    in_names.append(name)
            elif alloc.kind == "ExternalOutput":
                shape = tuple(alloc.tensor_shape)
                dtype = mybir.dt.np(alloc.dtype)
                out_avals.append(jax.core.ShapedArray(shape, dtype))
                out_names.append(name)
                zero_outs.append(np.zeros(shape, dtype))
        n_params = len(in_names)
        n_outs = len(out_names)
        all_in_names = in_names + out_names
        if pname is not None:
            all_in_names.append(pname)
        self.in_names = in_names
        self.out_names = out_names
        self.out_avals = out_avals
        self.zero_outs = zero_outs
        self.n_params = n_params

        def _body(*args):
            operands = list(args)
            if pname is not None:
                operands.append(partition_id_tensor())
            outs = _bass_exec_p.bind(
                *operands,
                out_avals=tuple(out_avals),
                in_names=tuple(all_in_names),
                out_names=tuple(out_names),
                lowering_input_output_aliases=(),
                sim_require_finite=True,
                sim_require_nnan=True,
                nc=nc,
            )
            return tuple(outs)

        devices = jax.devices()[:NCORES]
        self.mesh = Mesh(np.asarray(devices), ("core",))
        in_specs = (PartitionSpec("core"),) * (n_params + n_outs)
        out_specs = (PartitionSpec("core"),) * n_outs
        self.sharding = NamedSharding(self.mesh, PartitionSpec("core"))
        donate = tuple(range(n_params, n_params + n_outs))
        self.fn = jax.jit(
            shard_map(_body, mesh=self.mesh, in_specs=in_specs,
                      out_specs=out_specs, check_rep=False),
            donate_argnums=donate, keep_unused=True,
        )

    def concat_inputs(self, in_maps):
        return [
            np.concatenate([np.asarray(m[name]) for m in in_maps], axis=0)
            for name in self.in_names
        ]

    def device_put_inputs(self, in_maps):
        return [
            self.jax.device_put(a, self.sharding)
            for a in self.concat_inputs(in_maps)
        ]

    def call(self, concat_in):
        zeros = [
            np.zeros((NCORES * z.shape[0], *z.shape[1:]), z.dtype)
            for z in self.zero_outs
        ]
        return self.fn(*concat_in, *zeros)

    def run(self, in_maps):
        out_arrs = self.call(self.concat_inputs(in_maps))
        return [
            {
                name: np.asarray(out_arrs[i]).reshape(
                    NCORES, *self.out_avals[i].shape)[c]
                for i, name in enumerate(self.out_names)
            }
            for c in range(NCORES)
        ]


def _make_runner():
    if "runner" not in _CACHE:
        _CACHE["runner"] = _Runner(_get_program())
    return _CACHE["runner"]


def _in_maps(x, W, b):
    il, ih = _iota_tables()
    return [
        {
            "x": np.ascontiguousarray(x[i * B_LOC:(i + 1) * B_LOC]),
            "W": np.ascontiguousarray(W),
            "b": np.ascontiguousarray(b),
            "iota_lo": il,
            "iota_hi": ih,
        }
        for i in range(NCORES)
    ]


def _run(x, W, b, trace=False, trace_cores=None):
    from concourse.bass_utils import run_bass_kernel_spmd

    nc = _get_program()
    return run_bass_kernel_spmd(nc, _in_maps(x, W, b),
                                core_ids=list(range(NCORES)), trace=trace,
                                trace_cores=trace_cores)


def kernel(**inputs):
    x = inputs["x"]
    W = inputs["W"]
    b = inputs["b"]
    assert x.shape == (B_FULL, N, 3) and x.dtype == np.float32
    res = _make_runner().run(_in_maps(x, W, b))
    return np.concatenate([r["out"] for r in res], axis=0)
